# revision 23
# baseline (speedup 1.0000x reference)
"""Trainium2 Bass kernel for AttentionalFactorizationMachine (v2).

kernel(**inputs) takes FULL unsharded inputs, returns FULL [2048, 1] output.
Internally: data-parallel over 8 NeuronCores (batch sharded, weights
replicated), one SPMD Bass program.

Per-core algorithm (256 items, 780 pairs padded to 784):
  out[b] = (sum_p E_p * g_p) / (sum_p E_p) + fc_b
    E_p = exp(l_p)                       [proj_b dropped: softmax-invariant]
    l_p = proj_w . relu(attn_w^T (x_i*x_j) + attn_b)
    g_p = fc_w . (x_i*x_j)
  Device computes num/den per item; host does the final divide + fc_b.

v2 layout: logits and g are produced by matmuls with SMALL zero-padded
stationaries (weights stationary, data moving) instead of 128-col data
stationaries, eliminating the LDWEIGHTS bottleneck of v1. Outputs are
slot-packed: per 4-supertile block, psum tile lgg [128, 256] holds
  lgg[32*sl + 2*p_local + h, q]      sl=supertile-in-block
    cols 0:128  = logits,  cols 128:256 = g
so exp / E*g run on all 128 partitions. num/den are then partition-dim
reductions: matmuls with per-half selector stationaries accumulating into
a persistent psum tile nd_ps[2h+k, q] (k=0 num, k=1 den) across blocks.

Layouts (SBUF [partition, free]):
  X_T [ (half,d)=128, (field,b_q)=40*128 ] fp16   via DMA transpose
  ip  [ (half,d)=128, (pair_loc,b_q)     ] fp16   DVE broadcast tensor_mul (2x)
L1 per supertile (16 pairs): 8 concurrent matmuls (2 b-halves x 4
col-groups, tile_position), lhsT=attn_w -> ps_z [(cgrp,a)=128, 1024];
ACT fused bias+relu psum->SBUF H fp16.
"""

import numpy as np

B, F, D, A = 2048, 40, 64, 32
N_CORES = 8
BC = B // N_CORES          # 256 items per core
BQ = 128                   # items per half
N_HALF = 2
PAIRS = F * (F - 1) // 2   # 780
ST_PAIRS = 16
N_ST = (PAIRS + ST_PAIRS - 1) // ST_PAIRS       # 49
PAIRS_PAD = N_ST * ST_PAIRS                     # 784
BLK_ST = 4                                      # supertiles per slot-block
N_BLK = (N_ST + BLK_ST - 1) // BLK_ST           # 13 (12x4 + 1x1)
NEG_BIG = -1.0e30
IP_BUFS = 9
H_BUFS = 9
LG_SPLIT = 1      # 4 = split lg into 32-row tiles, 1 = full-row
G_SPLIT = False   # split g matmuls into per-half 64-row tiles

# pairs ordered by (j, i): enables field-chunked pipelining of x-prep
_ROWJ = np.array([i for j in range(1, F) for i in range(j)], np.int32)
_COLJ = np.array([j for j in range(1, F) for i in range(j)], np.int32)
N_FCHUNK = 4
CF = F // N_FCHUNK  # 10 fields per chunk

# ---- packed-constant column layout (wp16 [128, WP_W] fp16) ----
# aw:        cols 0:32, attn_w tiled twice vertically
# projstat:  8 variants (p0 in 4, h in 2), width 25+2*p0+h
#            col (8n + 2*p0 + h) = proj_w at rows 32n..32n+32
# gstat:     16 variants (p in 16), width 2p+2
#            col (2p + hh) = fc_w at rows 64*hh..64*hh+64
# ndstatG:   [128,4] col 2h   = 1.0 at slot rows with row%2==h  (num rows)
# ndstatE:   [128,4] col 2h+1 = 1.0 at slot rows with row%2==h  (den rows)
_OFF = {}
_w = 32
for _p0 in range(4):
    for _h in range(2):
        _OFF[("proj", _p0, _h)] = (_w, 25 + 2 * _p0 + _h)
        _w += 25 + 2 * _p0 + _h
for _p in range(16):
    _OFF[("g", _p)] = (_w, 2 * _p + 2)
    _w += 2 * _p + 2
_OFF[("ndG",)] = (_w, 4)
_w += 4
_OFF[("ndE",)] = (_w, 4)
_w += 4
_OFF[("zero",)] = (_w, 128)
_w += 128
WP_W = _w


def _patch_tile_drain():
    """This walrus build accepts only ONE sync wait per instruction; split the
    TileContext exit drain into a chain of single-wait drains."""
    import bass_rust
    import concourse.tile as tile_mod
    from concourse.tile import TileContext

    if getattr(TileContext, "_drain_patched", False):
        return

    def _drain_and_barrier(self, tick_clock, wait_clock):
        drain_inst = self.nc.sync.drain()
        wait_clock.add_sem_waits(
            drain_inst.ins, tile_mod.ScopedClock({None: tick_clock.global_clock})
        )
        si = drain_inst.ins.sync_info
        if si is not None and len(si.on_wait) > 1:
            waits = list(si.on_wait)
            drain_inst.ins.sync_info = bass_rust.SyncInfo(
                on_wait=[waits[0]], on_update=list(si.on_update)
            )
            for w in waits[1:]:
                extra = self.nc.sync.drain()
                extra.ins.sync_info = bass_rust.SyncInfo(on_wait=[w], on_update=[])

    TileContext._drain_and_barrier = _drain_and_barrier
    TileContext._drain_patched = True


def _split_multiwait(nc):
    """Walrus here allows ONE sync wait per instruction: move surplus waits
    onto same-engine NoOps inserted immediately before the instruction."""
    import concourse.mybir as mybir

    for f in nc.m.functions:
        for blk in f.blocks:
            il = blk.instructions
            idx = 0
            while idx < len(il):
                inst = il[idx]
                si = inst.sync_info
                if si is not None and len(si.on_wait) > 1:
                    waits = list(si.on_wait)
                    inst.sync_info = mybir.SyncInfo(
                        on_wait=[waits[-1]], on_update=list(si.on_update)
                    )
                    for k, w in enumerate(waits[:-1]):
                        nop = mybir.InstNoOp(
                            name=f"{inst.name}_w{k}",
                            sync_info=mybir.SyncInfo(on_wait=[w], on_update=[]),
                            bass_nofuse=True,
                            engine=inst.engine,
                        )
                        il.insert(idx, nop)
                        idx += 1
                idx += 1


def build_core_program(split_waits=True, repeat=1, skip=(), debug_blk=None):
    """The single-core SPMD Bass program (identical on all 8 cores)."""
    import concourse.bass as bass
    import concourse.mybir as mybir
    from concourse.tile import TileContext

    _patch_tile_drain()
    dt = mybir.dt
    AF = mybir.ActivationFunctionType
    ALU = mybir.AluOpType

    nc = bass.Bass()
    x_in = nc.dram_tensor("x", [BC, F, D], dt.float32, kind="ExternalInput")
    wp16_in = nc.dram_tensor("wp16", [128, WP_W], dt.float16, kind="ExternalInput")
    wp32_in = nc.dram_tensor("wp32", [128, 1], dt.float32, kind="ExternalInput")
    # rows (h, k): k=0 num, k=1 den; cols q
    out_t = nc.dram_tensor("out", [4, BQ], dt.float32, kind="ExternalOutput")
    dbg_t = None
    if debug_blk is not None:
        dbg_t = nc.dram_tensor("dbg", [2, 128, BQ], dt.float32,
                               kind="ExternalOutput")

    # fp16 field-major scratch for DMA transpose: [f, b_q, (h, d)]
    scratch = nc.dram_tensor("scratch", [F, BQ, N_HALF, D], dt.float16)

    # per-supertile segments: ("tt", j, i0, i1, p_local0) or ("pad", _, _, n, pl)
    seg_of_st = []
    for s in range(N_ST):
        p_lo, p_hi = s * ST_PAIRS, (s + 1) * ST_PAIRS
        segs, p = [], p_lo
        while p < p_hi:
            if p < PAIRS:
                i, j = int(_ROWJ[p]), int(_COLJ[p])
                run = min(p_hi, PAIRS, p + (j - i)) - p
                i_end = i + run
                while i < i_end:
                    i_stop = min(i_end, (i // CF + 1) * CF)
                    segs.append(("tt", j, i, i_stop, p - p_lo))
                    p += i_stop - i
                    i = i_stop
            else:
                segs.append(("pad", 0, 0, p_hi - p, p - p_lo))
                p = p_hi
        seg_of_st.append(segs)

    with TileContext(nc) as tc:
        with (
            tc.tile_pool(name="const", bufs=1) as cpool,
            tc.tile_pool(name="xstage", bufs=1) as xpool,
            tc.tile_pool(name="ip", bufs=IP_BUFS) as ippool,
            tc.tile_pool(name="hbuf", bufs=H_BUFS) as hpool,
            tc.tile_pool(name="esb", bufs=2) as epool,
            tc.tile_pool(name="pz", bufs=2, space="PSUM") as pz,
            tc.tile_pool(name="plgg", bufs=2, space="PSUM") as plgg,
            tc.tile_pool(name="pnd", bufs=1, space="PSUM") as pnd,
        ):
            import contextlib
            loop_cm = (tc.For_i(0, repeat, 1) if repeat > 1
                       else contextlib.nullcontext())
            with loop_cm:
                # ---------- weights prep (host-packed, 2 DMAs) ----------
                wp16 = cpool.tile([128, WP_W], dt.float16)
                nc.sync.dma_start(wp16[:], wp16_in[:])
                ab = cpool.tile([128, 1], dt.float32)
                nc.sync.dma_start(ab[:], wp32_in[:])
                aw = wp16[:, 0:A]

                def stat(key):
                    off, w = _OFF[key]
                    return wp16[:, off:off + w]

                # ---------- x prep: 4 field-chunks, pipelined ----------
                xt_tiles = []
                for fc in range(N_FCHUNK):
                    f0, f1 = fc * CF, (fc + 1) * CF
                    x_f32 = xpool.tile(
                        [BQ, N_HALF * CF * D], dt.float32,
                        tag="xf32", name=f"x_f32_{fc}",
                    )
                    nc.sync.dma_start(
                        x_f32[:].rearrange(
                            "q (h f d) -> q h f d", h=N_HALF, f=CF),
                        x_in[:, f0:f1, :].rearrange(
                            "(h q) f d -> q h f d", h=N_HALF),
                    )
                    x_f16 = xpool.tile(
                        [BQ, N_HALF * CF * D], dt.float16,
                        tag="xf16", name=f"x_f16_{fc}",
                    )
                    nc.vector.tensor_copy(x_f16[:], x_f32[:])
                    nc.scalar.dma_start(
                        scratch[f0:f1].rearrange("f q h d -> q h f d"),
                        x_f16[:].rearrange(
                            "q (h f d) -> q h f d", h=N_HALF, f=CF),
                    )
                    xtc = xpool.tile(
                        [128, CF * BQ], dt.float16,
                        tag=f"xt{fc}", name=f"xt_{fc}",
                    )
                    nc.sync.dma_start_transpose(
                        xtc[:],
                        scratch[f0:f1].rearrange("f q h d -> (f q) (h d)"),
                    )
                    xt_tiles.append(xtc)

                # persistent num/den psum accumulator: rows (h,k) k=0 num,1 den
                nd_ps = pnd.tile([4, BQ], dt.float32)

                def phase_a(b):
                    """ip build + L1 + relu for each supertile of block b."""
                    st0, st1 = b * BLK_ST, min((b + 1) * BLK_ST, N_ST)
                    ips, hss = [], []
                    for s in range(st0, st1):
                        # ---- ip build ----
                        ip = ippool.tile([128, ST_PAIRS * BQ], dt.float16,
                                         tag="ip")
                        if "ip" in skip:
                            x0 = xt_tiles[0]
                            nc.vector.tensor_mul(
                                ip[:, 0:BQ], x0[:, 0:BQ], x0[:, BQ:2 * BQ])
                        for kind, j, i0, i1, pl in (
                                seg_of_st[s] if "ip" not in skip else []):
                            if kind == "pad":
                                npd = i1 - i0
                                nc.vector.memset(
                                    ip[:, pl * BQ:(pl + npd) * BQ], 0.0)
                                continue
                            nii = i1 - i0
                            xt_i = xt_tiles[i0 // CF]
                            xt_j = xt_tiles[j // CF]
                            il, jl = i0 % CF, j % CF
                            nc.vector.tensor_mul(
                                ip[:, pl * BQ:(pl + nii) * BQ].rearrange(
                                    "p (j q) -> p j q", j=nii
                                ),
                                xt_i[:, il * BQ:(il + nii) * BQ].rearrange(
                                    "p (j q) -> p j q", j=nii
                                ),
                                xt_j[:, jl * BQ:(jl + 1) * BQ].rearrange(
                                    "p (o q) -> p o q", o=1
                                ).broadcast_to((128, nii, BQ)),
                            )
                        ips.append(ip)

                        # ---- L1: 8 concurrent matmuls, one 2-bank tile ----
                        ps_z = pz.tile([128, 1024], dt.float32, tag="z",
                                       name=f"ps_z_{s}")
                        for h in (range(N_HALF) if "l1" not in skip else []):
                            for c in range(4):
                                nc.tensor.matmul(
                                    ps_z[32 * c:32 * (c + 1),
                                         512 * h:512 * (h + 1)],
                                    aw[64 * h:64 * (h + 1), :],
                                    ip[64 * h:64 * (h + 1),
                                       512 * c:512 * (c + 1)],
                                    start=True, stop=True,
                                    tile_position=(64 * h, 32 * c),
                                )

                        # ---- relu + bias ----
                        hs = hpool.tile([128, 1024], dt.float16, tag="h",
                                        name=f"hs_{s}")
                        if "relu" in skip:
                            nc.scalar.activation(
                                hs[:, 0:16], ps_z[:, 0:16], AF.Relu,
                                bias=ab[:], scale=1.0)
                        else:
                            nc.scalar.activation(
                                hs[:], ps_z[:], AF.Relu, bias=ab[:],
                                scale=1.0)
                        hss.append(hs)
                    return ips, hss

                def phase_b(b, ips, hss):
                    """slot matmuls + exp/EG/nd for block b (issued one
                    block behind phase_a so ACT relu of block b+1 overlaps
                    the PE slot matmuls of block b)."""
                    st0, st1 = b * BLK_ST, min((b + 1) * BLK_ST, N_ST)
                    # full bank so the 2 pool bufs never share a psum bank;
                    # only cols 0:256 are used (0:128 logits, 128:256 g)
                    lgg = plgg.tile([128, 512], dt.float32, tag="lgg")
                    # zero-wall: one full-array matmul with an all-zero
                    # stationary zeroes cols 0:256 and claims every element
                    # (start=True); real slot matmuls below overlap it, so
                    # they are ordered after it and accumulate (start=False)
                    # regardless of scheduler order.
                    zoff, _zw = _OFF[("zero",)]
                    nc.tensor.matmul(
                        lgg[:, 0:256], wp16[:, zoff:zoff + 128],
                        wp16[:, 0:256],
                        start=True, stop=False, skip_group_check=True,
                    )

                    # slot matmuls. Every matmul is a ROW-DISJOINT tile
                    # (32- or 64-row contraction band) so LDWEIGHTS of one
                    # tile overlaps in-flight matmuls of other bands, and
                    # streams of different (row, col) tiles run concurrently.
                    # g for pair p, half hh: contraction rows 64hh..64hh+64,
                    #   stationary col (2p+hh) -> slot 32sl+2p+hh.
                    # lg for (p0, h, n): contraction rows 32n..32n+32 (c=n),
                    #   stationary col (8n+2p0+h) -> slot 32sl+8n+2p0+h.
                    # All accumulate (start=False) under the zero-wall.
                    ops_of_sl = []
                    for s in range(st0, st1):
                        sl = s - st0
                        lg_atoms, g_atoms = [], []
                        if "l2" not in skip:
                            for p0 in range(4):
                                for h in range(N_HALF):
                                    for n in range(LG_SPLIT):
                                        lg_atoms.append(("lg", p0, h, n))
                        if "g" not in skip:
                            for p in range(15, -1, -1):
                                for hh in (range(N_HALF) if G_SPLIT
                                           else (0,)):
                                    g_atoms.append(("g", p, hh, 0))
                        # interleave lg (32-row bands) with g (64-row bands)
                        ops = []
                        li = gi = 0
                        while li < len(lg_atoms) or gi < len(g_atoms):
                            if gi < len(g_atoms):
                                ops.append(g_atoms[gi]); gi += 1
                            if li < len(lg_atoms):
                                ops.append(lg_atoms[li]); li += 1
                            if li < len(lg_atoms):
                                ops.append(lg_atoms[li]); li += 1
                        ops_of_sl.append((sl, ops))
                    max_ops = max((len(o) for _, o in ops_of_sl), default=0)
                    n_ops_tot = sum(len(o) for _, o in ops_of_sl)
                    op_no = 0
                    for k in range(max_ops):
                        for sl, ops in ops_of_sl:
                            if k >= len(ops):
                                continue
                            kind, a, b_, n = ops[k]
                            is_stop = op_no == n_ops_tot - 1
                            op_no += 1
                            if kind == "lg":
                                p0, h = a, b_
                                off, w0 = _OFF[("proj", p0, h)]
                                if LG_SPLIT == 4:
                                    w = 8 * n + 2 * p0 + h + 1
                                    nc.tensor.matmul(
                                        lgg[32 * sl:32 * sl + w, 0:BQ],
                                        wp16[32 * n:32 * (n + 1),
                                             off:off + w],
                                        hss[sl][32 * n:32 * (n + 1),
                                                512 * h + 128 * p0:
                                                512 * h + 128 * (p0 + 1)],
                                        start=False, stop=is_stop,
                                        tile_position=(32 * n, 32 * sl),
                                        skip_group_check=True,
                                    )
                                else:
                                    nc.tensor.matmul(
                                        lgg[32 * sl:32 * sl + w0, 0:BQ],
                                        wp16[:, off:off + w0],
                                        hss[sl][:, 512 * h + 128 * p0:
                                                512 * h + 128 * (p0 + 1)],
                                        start=False, stop=is_stop,
                                        tile_position=(0, 32 * sl),
                                        skip_group_check=True,
                                    )
                            else:
                                p, hh = a, b_
                                off, w0 = _OFF[("g", p)]
                                if G_SPLIT:
                                    w = 2 * p + hh + 1
                                    nc.tensor.matmul(
                                        lgg[32 * sl:32 * sl + w,
                                            128:128 + BQ],
                                        wp16[64 * hh:64 * (hh + 1),
                                             off:off + w],
                                        ips[sl][64 * hh:64 * (hh + 1),
                                                BQ * p:BQ * (p + 1)],
                                        start=False, stop=is_stop,
                                        tile_position=(64 * hh, 32 * sl),
                                        skip_group_check=True,
                                    )
                                else:
                                    nc.tensor.matmul(
                                        lgg[32 * sl:32 * sl + w0,
                                            128:128 + BQ],
                                        wp16[:, off:off + w0],
                                        ips[sl][:, BQ * p:BQ * (p + 1)],
                                        start=False, stop=is_stop,
                                        tile_position=(0, 32 * sl),
                                        skip_group_check=True,
                                    )

                    # ---- poison unused slots (last block only) ----
                    # (pad pairs 780..783 keep E=exp(c0) with g=0; the host
                    # subtracts 4*exp(c0) from den -- psum memsets must be
                    # 32-partition aligned so they can't be poisoned here)
                    n_st_b = st1 - st0
                    if st1 == N_ST:
                        for g0 in range(n_st_b, BLK_ST):
                            nc.vector.memset(
                                lgg[32 * g0:32 * (g0 + 1), 0:BQ], NEG_BIG)
                            nc.vector.memset(
                                lgg[32 * g0:32 * (g0 + 1), 128:256], 0.0)

                    # ---- exp -> E, E*g -> EG, nd accumulation ----
                    e_sb = epool.tile([128, BQ], dt.float16, tag="E")
                    nc.scalar.activation(e_sb[:], lgg[:, 0:BQ], AF.Exp)
                    eg_sb = epool.tile([128, BQ], dt.float16, tag="EG")
                    nc.vector.tensor_mul(eg_sb[:], e_sb[:], lgg[:, 128:256])
                    if debug_blk == b:
                        dE = epool.tile([128, BQ], dt.float32, tag="dbgE")
                        nc.vector.tensor_copy(dE[:], lgg[:, 0:BQ])
                        nc.scalar.dma_start(dbg_t[0], dE[:])
                        dG = epool.tile([128, BQ], dt.float32, tag="dbgG")
                        nc.vector.tensor_copy(dG[:], lgg[:, 128:256])
                        nc.scalar.dma_start(dbg_t[1], dG[:])
                    nc.tensor.matmul(
                        nd_ps[:], stat(("ndG",)), eg_sb[:],
                        start=(b == 0), stop=False,
                        skip_group_check=True,
                    )
                    nc.tensor.matmul(
                        nd_ps[:], stat(("ndE",)), e_sb[:],
                        start=False, stop=(b == N_BLK - 1),
                        skip_group_check=True,
                    )

                # 2-stage software pipeline over blocks
                pend = None
                for b in range(N_BLK):
                    cur = (b, *phase_a(b))
                    if pend is not None:
                        phase_b(*pend)
                    pend = cur
                phase_b(*pend)

                # ---------- epilogue: emit num/den ----------
                nd_sb = epool.tile([4, BQ], dt.float32, tag="ndsb")
                nc.vector.tensor_copy(nd_sb[:], nd_ps[:])
                nc.sync.dma_start(out_t[:], nd_sb[:])

    if split_waits:
        _split_multiwait(nc)
    return nc


def pack_weights(attn_w, attn_b, proj_w, fc_w):
    """Host-side packing of the tiny weights into device-ready layouts."""
    attn_w = np.asarray(attn_w, np.float32)
    attn_b = np.asarray(attn_b, np.float32).reshape(A)
    proj_w = np.asarray(proj_w, np.float32).reshape(A)
    fc_w = np.asarray(fc_w, np.float32).reshape(D)
    wp16 = np.zeros((128, WP_W), np.float16)
    wp16[0:D, 0:A] = attn_w.astype(np.float16)
    wp16[D:2 * D, 0:A] = attn_w.astype(np.float16)
    for p0 in range(4):
        for h in range(2):
            off, w = _OFF[("proj", p0, h)]
            for n in range(4):
                c = 8 * n + 2 * p0 + h
                wp16[32 * n:32 * (n + 1), off + c] = proj_w.astype(np.float16)
    for p in range(16):
        off, w = _OFF[("g", p)]
        for hh in range(2):
            wp16[64 * hh:64 * (hh + 1), off + 2 * p + hh] = \
                fc_w.astype(np.float16)
    offG, _ = _OFF[("ndG",)]
    offE, _ = _OFF[("ndE",)]
    rows = np.arange(128)
    for h in range(2):
        wp16[:, offG + 2 * h] = (rows % 2 == h).astype(np.float16)
        wp16[:, offE + 2 * h + 1] = (rows % 2 == h).astype(np.float16)
    wp32 = np.tile(attn_b, 4).reshape(128, 1).astype(np.float32)
    return wp16, wp32


_CACHED = {}


def _get_runner():
    if "runner" in _CACHED:
        return _CACHED["runner"]
    import jax
    from jax.sharding import Mesh, PartitionSpec
    from jax.experimental.shard_map import shard_map
    import concourse.mybir as mybir
    from concourse.bass2jax import (
        _bass_exec_p, install_neuronx_cc_hook, partition_id_tensor,
    )

    nc = build_core_program()
    install_neuronx_cc_hook()

    partition_name = nc.partition_id_tensor.name if nc.partition_id_tensor else None
    in_names, out_names, out_avals, zero_outs = [], [], [], []
    for alloc in nc.m.functions[0].allocations:
        if not isinstance(alloc, mybir.MemoryLocationSet):
            continue
        name = alloc.memorylocations[0].name
        if alloc.kind == "ExternalInput":
            if name != partition_name:
                in_names.append(name)
        elif alloc.kind == "ExternalOutput":
            out_names.append(name)
            shape = tuple(alloc.tensor_shape)
            dtype = mybir.dt.np(alloc.dtype)
            out_avals.append(jax.core.ShapedArray(shape, dtype))
            zero_outs.append(np.zeros(shape, dtype))
    n_params = len(in_names)
    n_outs = len(out_avals)
    all_in = in_names + out_names + ([partition_name] if partition_name else [])

    def _body(*args):
        operands = list(args)
        if partition_name is not None:
            operands.append(partition_id_tensor())
        outs = _bass_exec_p.bind(
            *operands,
            out_avals=tuple(out_avals),
            in_names=tuple(all_in),
            out_names=tuple(out_names),
            lowering_input_output_aliases=(),
            sim_require_finite=True,
            sim_require_nnan=True,
            nc=nc,
        )
        return tuple(outs)

    devices = jax.devices()[:N_CORES]
    mesh = Mesh(np.asarray(devices), ("core",))
    fn = jax.jit(
        shard_map(
            _body, mesh=mesh,
            in_specs=(PartitionSpec("core"),) * (n_params + n_outs),
            out_specs=(PartitionSpec("core"),) * n_outs,
            check_rep=False,
        ),
        keep_unused=True,
    )
    _CACHED["runner"] = {
        "fn": fn, "in_names": in_names, "out_names": out_names,
        "zero_outs": zero_outs, "mesh": mesh, "nc": nc,
    }
    return _CACHED["runner"]


def _device_args(r, x, attn_w, attn_b, proj_w, fc_w):
    wp16, wp32 = pack_weights(attn_w, attn_b, proj_w, fc_w)
    feeds = {
        "x": np.ascontiguousarray(
            np.asarray(x, np.float32).reshape(N_CORES * BC, F, D)
        ),
        "wp16": np.ascontiguousarray(np.tile(wp16, (N_CORES, 1))),
        "wp32": np.ascontiguousarray(np.tile(wp32, (N_CORES, 1))),
    }
    concat_in = [feeds[n] for n in r["in_names"]]
    concat_zeros = [
        np.zeros((N_CORES * z.shape[0], *z.shape[1:]), z.dtype)
        for z in r["zero_outs"]
    ]
    return concat_in + concat_zeros


def kernel(x, attn_w, attn_b, proj_w, proj_b, fc_w, fc_b):
    """FULL inputs -> FULL output. proj_b is softmax-invariant (unused)."""
    import jax

    r = _get_runner()
    args = _device_args(r, x, attn_w, attn_b, proj_w, fc_w)
    outs = r["fn"](*args)
    jax.block_until_ready(outs)
    nd = np.asarray(outs[r["out_names"].index("out")]).reshape(
        N_CORES, 2, 2, BQ)  # [core, h, k, q]
    num = nd[:, :, 0, :].reshape(B)
    den = nd[:, :, 1, :].reshape(B)
    # pad pairs (780..783): ip=0 on device, so g=0 (num unaffected) but
    # E=exp(c0) with c0 = proj_w . relu(attn_b); remove them from den.
    # Mirror device fp16 weight rounding.
    ab16 = np.asarray(attn_b, np.float32).reshape(A).astype(np.float16)
    pw16 = np.asarray(proj_w, np.float32).reshape(A).astype(np.float16)
    c0 = float(np.dot(pw16.astype(np.float32),
                      np.maximum(ab16.astype(np.float32), 0.0)))
    den = den - (PAIRS_PAD - PAIRS) * np.exp(np.float32(c0))
    fc_b = np.asarray(fc_b, np.float32)
    return (num / den + fc_b[0]).astype(np.float32)[:, None]
